# revision 2
# baseline (speedup 1.0000x reference)
"""Combined focal + MDCA loss kernel for Trainium2 (8 NeuronCores, SPMD) — v5.

Device keeps only the O(B*C) work:
  - exp of every logit (ACT engine; one wide ACTIVATE per group for most
    tiles, per-tile ACTIVATE+accumulator for ACC tiles per group)
  - per-row softmax denominators s (ACT accumulator for ACC tiles, DVE
    tensor_scalar cache-reduce for the rest)
  - per-class confidence sums conf_c = sum_rows e[r,c]/s[r] as fp16
    matmuls (1/s)^T @ e accumulated in PSUM across all 128 tiles.

Everything O(B) moved to the host combine step (it is finalize-scale work,
per the sharding hint "psum of partial sums then finalize"):
  - counts_c = bincount(targets)
  - focal term from the device row-sums s and the (fp16-rounded) target
    logit x_t: logpt = x_t - log s; focal = mean((1-pt)^2 * -logpt)
This removes the v4 one-hot gather (43 us DVE), the counts matmuls
(~55 us PE), the et clamps/reciprocals, and the Ln/Square focal finalize
(extra ACT tables + ops). No row sorting is needed anymore.

Engine budget per core (measured v4 rates): ACT = 112 wide-exp tiles
(~0.91 us each) + 16 accum-exp tiles (~1.38 us) ~= 124 us; DVE = 112
cache-reduces (~1.19 us) + group reciprocals ~= 137 us; PE = 256 conf
matmuls ~= 95 us; DMA-in 32.75 MB fp16 ~= 95-110 us. All overlap.
"""

import numpy as np

import bass_rust
import concourse.bass as bass
import concourse.tile as tile
from concourse import mybir
from concourse.bass_utils import run_bass_kernel_spmd

N_CORES = 8
B, C = 131072, 1000
ROWS = B // N_CORES  # rows per core
P = 128              # partitions (batch rows per tile)
NT = ROWS // P       # tiles per core
GAMMA = 2.0
BETA = 5.0
NSPLIT = 512         # PSUM bank / matmul free-dim split of C
GRP = 8              # tiles per DMA group
ACC = 1              # tiles per group using the ACT accumulator for s


def _split_excess_waits(nc, max_waits=1):
    """walrus on this path encodes at most one sync-wait per instruction;
    hoist extras onto EventSemaphore instructions on the same engine."""
    for bbb in nc.bb_map.values():
        bb = bbb.bb
        insts = list(bb.instructions)
        out = []
        changed = False
        for ins in insts:
            si = ins.sync_info
            if si is not None and len(si.on_wait) > max_waits:
                waits = list(si.on_wait)
                for w in waits[max_waits:]:
                    ev = mybir.InstEventSemaphore(
                        name=nc.get_next_instruction_name(), ins=[], outs=[]
                    )
                    ev.engine = ins.engine
                    ev.sync_info = bass_rust.SyncInfo(on_wait=[w], on_update=[])
                    try:
                        nc.register_instruction(ev)
                    except Exception:
                        pass
                    out.append(ev)
                si.on_wait = waits[:max_waits]
                changed = True
            out.append(ins)
        if changed:
            bb.instructions = out


def build(rows=ROWS, in_bufs=4, work_bufs=6, wide_bufs=4):
    nt = rows // P
    f32 = mybir.dt.float32
    f16 = mybir.dt.float16
    AF = mybir.ActivationFunctionType
    OP = mybir.AluOpType
    grp = min(GRP, nt)
    assert nt % grp == 0
    wide = grp - ACC

    nc = bass.Bass()
    # host-relaid fp16: lgr[p, i*C:(i+1)*C] = shard_logits[i*P+p]
    lgr = nc.dram_tensor("logits", [P, nt * C], f16, kind="ExternalInput")
    out_vec = nc.dram_tensor("out_vec", [1, C], f32, kind="ExternalOutput")
    out_s = nc.dram_tensor("s_out", [P, nt], f32, kind="ExternalOutput")

    with tile.TileContext(nc) as tc:
        with (
            tc.tile_pool(name="singles", bufs=1) as singles,
            tc.tile_pool(name="inp", bufs=in_bufs) as inp,
            tc.tile_pool(name="ework", bufs=work_bufs) as ework,
            tc.tile_pool(name="wwork", bufs=wide_bufs) as wwork,
            tc.tile_pool(name="psum", bufs=1, space="PSUM") as psum,
        ):
            s_cols = singles.tile([P, nt], f32)
            rs16 = singles.tile([P, nt], f16)
            sjunk = singles.tile([P, C], f16)   # cache-reduce dump target

            conf_ps = [
                psum.tile([1, NSPLIT], f32, name="conf0"),
                psum.tile([1, C - NSPLIT], f32, name="conf1"),
            ]

            e_tiles = {}
            for g in range(nt // grp):
                ltg = inp.tile([P, grp * C], f16)
                nc.sync.dma_start(
                    out=ltg, in_=lgr[:, g * grp * C : (g + 1) * grp * C]
                )
                base = g * grp
                # tiles 0..ACC-1: per-tile exp with ACT accumulator -> s
                for j in range(ACC):
                    i = base + j
                    e = ework.tile([P, C], f16)
                    nc.scalar.activation(
                        out=e,
                        in_=ltg[:, j * C : (j + 1) * C],
                        func=AF.Exp,
                        accum_out=s_cols[:, i : i + 1],
                    )
                    e_tiles[i] = e
                # tiles ACC..grp-1: one wide exp, s via DVE cache-reduce
                ew = wwork.tile([P, wide * C], f16)
                nc.scalar.activation(
                    out=ew, in_=ltg[:, ACC * C : grp * C], func=AF.Exp
                )
                for j in range(ACC, grp):
                    i = base + j
                    sl = ew[:, (j - ACC) * C : (j - ACC + 1) * C]
                    e_tiles[i] = sl
                    nc.vector.tensor_scalar(
                        out=sjunk,
                        in0=sl,
                        scalar1=1.0,
                        scalar2=0.0,
                        op0=OP.mult,
                        op1=OP.add,
                        accum_out=s_cols[:, i : i + 1],
                    )
                # batched reciprocal for the group
                sl = slice(base, base + grp)
                with nc.allow_low_precision(
                    reason="fp16 matmul operands; errors average over 131k rows"
                ):
                    nc.vector.reciprocal(out=rs16[:, sl], in_=s_cols[:, sl])
                # conf matmuls for the group
                for j in range(grp):
                    i = base + j
                    first, last = i == 0, i == nt - 1
                    ek = e_tiles.pop(i)
                    rk = rs16[:, i : i + 1]
                    nc.tensor.matmul(
                        conf_ps[0], rk, ek[:, :NSPLIT], start=first, stop=last
                    )
                    nc.tensor.matmul(
                        conf_ps[1], rk, ek[:, NSPLIT:], start=first, stop=last
                    )

            # ---- outputs: conf PSUM -> SBUF -> DRAM, row sums -> DRAM ----
            ov = singles.tile([1, C], f32)
            nc.scalar.copy(out=ov[:, :NSPLIT], in_=conf_ps[0])
            nc.scalar.copy(out=ov[:, NSPLIT:], in_=conf_ps[1])
            nc.sync.dma_start(out=out_vec[:], in_=ov)
            nc.sync.dma_start(out=out_s[:], in_=s_cols)

    _split_excess_waits(nc)
    return nc


_NC_CACHE = {}


def _get_nc():
    if "nc" not in _NC_CACHE:
        _NC_CACHE["nc"] = build()
    return _NC_CACHE["nc"]


def make_in_maps(logits):
    logits = np.asarray(logits, dtype=np.float32)
    nt = ROWS // P
    in_maps = []
    for c in range(N_CORES):
        lsh = logits[c * ROWS : (c + 1) * ROWS]
        lr = np.ascontiguousarray(
            lsh.reshape(nt, P, C).transpose(1, 0, 2).reshape(P, nt * C)
        ).astype(np.float16)
        in_maps.append({"logits": lr})
    return in_maps


def combine(results, logits, targets):
    logits = np.asarray(logits, dtype=np.float32)
    targets = np.asarray(targets).astype(np.int64)
    nt = ROWS // P

    conf = np.zeros(C, np.float64)
    focal_sum = 0.0
    for c, r in enumerate(results):
        conf += r["out_vec"][0].astype(np.float64)
        # s_out[p, i] is the row-sum of shard row i*P + p
        s = r["s_out"].astype(np.float64).T.reshape(-1)  # [ROWS]
        tsh = targets[c * ROWS : (c + 1) * ROWS]
        lsh = logits[c * ROWS : (c + 1) * ROWS]
        # device consumed fp16 logits; use the same rounding for x_t
        x_t = (
            lsh[np.arange(ROWS), tsh].astype(np.float16).astype(np.float64)
        )
        logpt = x_t - np.log(s)
        pt = np.exp(logpt)
        focal_sum += ((1.0 - pt) ** GAMMA * logpt).sum()

    cnt = np.bincount(targets, minlength=C).astype(np.float64)
    loss_focal = -focal_sum / B
    loss_mdca = np.abs(conf / B - cnt / B).mean()
    return np.float32(loss_focal + BETA * loss_mdca)


def kernel(logits, targets):
    nc = _get_nc()
    in_maps = make_in_maps(logits)
    res = run_bass_kernel_spmd(nc, in_maps, list(range(N_CORES)))
    return combine(res.results, logits, targets)
